# Initial kernel scaffold
#
"""Trainium2 Bass kernel for a dense transformer block (cross-attn + RoPE
self-attn + SwiGLU MLP), SPMD over 8 NeuronCores.

Sharding: core = (batch, half). Each core processes one batch (B=4) and half
its tokens (balanced causal split: blocks {0,1,6,7} vs {2,3,4,5} of 8x128).
Cross-attention and self-attention K/V are computed for the full sequence on
both cores of a pair (cheap duplication, no collectives). Q / attention /
output-proj / MLP run only on the core's own 512 tokens.

All activations are feature-major [C, tokens]; matmuls run in bf16 with fp32
accumulation; the residual stream stays fp32. Host-side weight prep folds the
RMSNorm gammas and the attention scale into the weights, pre-transposes them,
and applies a rope-deinterleave permutation to the self-attn q/k weights.
The even/odd-core differences (softmax masks, own-token strip offsets) are
shipped as per-core data so a single SPMD program serves all 8 cores.
"""

import numpy as np
import ml_dtypes

import concourse.bacc as bacc
import concourse.bass as bass
import concourse.mybir as mybir
import concourse.tile as tile
from concourse import bass_utils
from concourse.bass import ds

F32 = mybir.dt.float32
BF16 = mybir.dt.bfloat16
AF = mybir.ActivationFunctionType
ALU = mybir.AluOpType

B, T, M, C, H, FF = 4, 1024, 256, 1024, 16, 4096
HD = C // H
EPS = 1e-5
N_CORES = 8
P = 128
CT = C // P            # 8 c-tiles
TT = T // P            # 8 token blocks
T_OWN = T // 2         # 512 own tokens per core
OT = T_OWN // P        # 4 own blocks
FT = FF // P           # 32 ff tiles
MT = M // P            # 2 memory tiles (cross keys)
NEG = -1e30

# Own token blocks per half (causally balanced: 1+2+7+8 == 3+4+5+6 == 18)
OWN_BLOCKS = {0: [0, 1, 6, 7], 1: [2, 3, 4, 5]}
# Own tokens as two contiguous 256-col strips (start offsets)
STRIPS = {0: [0, 768], 1: [256, 512]}
# Self-attn slots: identical shapes on both halves. Slot i processes own block
# OWN_BLOCKS[h][i] with klen = KLEN[i]; the shipped mask covers MASK_RANGE[i].
KLEN = [384, 512, 896, 1024]
MASK_RANGE = [(0, 384), (128, 512), (512, 896), (640, 1024)]

_CACHE = {}


def _bf16(a):
    return np.ascontiguousarray(a.astype(ml_dtypes.bfloat16))


def _f32(a):
    return np.ascontiguousarray(a.astype(np.float32))


def rope_perm():
    """Row permutation for self-attn q/k weights: per head, even hd indices
    first (rows h*32+j <- h*64+2j), all heads' real parts in rows 0:512,
    imag parts in rows 512:1024."""
    perm = np.zeros(C, dtype=np.int64)
    for h in range(H):
        for j in range(HD // 2):
            perm[h * (HD // 2) + j] = h * HD + 2 * j
            perm[C // 2 + h * (HD // 2) + j] = h * HD + 2 * j + 1
    return perm


def build_program():
    nc = bacc.Bacc("TRN2", target_bir_lowering=False, debug=False,
                   num_devices=N_CORES)

    def din(name, shape, dtype):
        return nc.dram_tensor(name, shape, dtype, kind="ExternalInput").ap()

    xT = din("xT", [C, T], F32)
    yT = din("yT", [C, M], F32)
    ca_wqT = din("ca_wqT", [C, C], BF16)
    ca_wkT = din("ca_wkT", [C, C], BF16)
    ca_wvT = din("ca_wvT", [C, C], BF16)
    ca_woT = din("ca_woT", [C, C], BF16)
    sa_wqT = din("sa_wqT", [C, C], BF16)
    sa_wkT = din("sa_wkT", [C, C], BF16)
    sa_wvT = din("sa_wvT", [C, C], BF16)
    sa_woT = din("sa_woT", [C, C], BF16)
    w_fc1T = din("w_fc1T", [C, FF], BF16)
    w_fc2T = din("w_fc2T", [C, FF], BF16)
    w_projT = din("w_projT", [FF, C], BF16)
    cosrep = din("cosrep", [P, T], F32)
    sinrep = din("sinrep", [P, T], F32)
    smask = din("smask", [OT, P, 384], F32)
    strips = din("strips", [1, 2], mybir.dt.uint32)
    outT = nc.dram_tensor("outT", [C, T_OWN], F32, kind="ExternalOutput").ap()

    with tile.TileContext(nc) as tc:
        _body(tc, locals())
    nc.compile()
    return nc


def _body(tc, io):
    nc = tc.nc

    # ---- global pools / constants ----
    const = tc.alloc_tile_pool(name="const", bufs=1)
    ones = const.tile([P, 1], BF16)
    nc.vector.memset(ones, 1.0)
    eps_t = const.tile([1, 1], F32)
    nc.vector.memset(eps_t, EPS)
    cs = const.tile([P, T], F32, tag="cos")
    sn = const.tile([P, T], F32, tag="sin")
    nc.gpsimd.dma_start(out=cs, in_=io["cosrep"])
    nc.gpsimd.dma_start(out=sn, in_=io["sinrep"])
    stile = const.tile([1, 2], mybir.dt.uint32)
    nc.gpsimd.dma_start(out=stile, in_=io["strips"])
    masks = [const.tile([P, 384], F32, tag=f"smask{s}", name=f"smask{s}")
             for s in range(OT)]
    for s in range(OT):
        nc.gpsimd.dma_start(out=masks[s], in_=io["smask"][s])

    wpool = tc.alloc_tile_pool(name="w", bufs=1)
    pspool = tc.alloc_tile_pool(name="ps", bufs=1, space="PSUM")
    normp = tc.alloc_tile_pool(name="normp", bufs=1)
    xopool = tc.alloc_tile_pool(name="xo", bufs=1)
    xo = [xopool.tile([P, T_OWN], F32, tag=f"xo_{c}", name=f"xo_{c}")
          for c in range(CT)]
    attnp = tc.alloc_tile_pool(name="attnp", bufs=1)

    def wtile():
        return wpool.tile([P, P], BF16, tag="wt", name="wt", bufs=40)

    def wload_fused(wT, o, ntiles=None, ocols=P):
        """One DMA for all CT contraction tiles of output cols [o*ocols, +ocols).
        Returns tile [P, ntiles, ocols]; lhsT for c-tile c is t[:, c, :]."""
        nt = CT if ntiles is None else ntiles
        wtb = wpool.tile([P, nt, ocols], BF16, tag=f"wtb{nt}_{ocols}",
                         name="wtb", bufs=8 if nt == CT and ocols == P else 2)
        src = wT[:, o * ocols:(o + 1) * ocols].rearrange(
            "(a p) o -> p a o", p=P)
        nc.sync.dma_start(out=wtb, in_=src)
        return wtb

    def rmsnorm(pool, src, ncols, tag):
        out = [pool.tile([P, ncols], BF16, tag=f"xn_{tag}_{c}",
                         name=f"xn_{tag}_{c}") for c in range(CT)]
        for n0 in range(0, ncols, 512):
            nn = min(512, ncols - n0)
            ssq = pspool.tile([1, nn], F32, tag="ohead", name="ssq", bufs=2)
            for c in range(CT):
                sq = normp.tile([P, nn], BF16, tag="sq", name="sq", bufs=4)
                if c % 2 == 0:
                    nc.vector.tensor_mul(out=sq, in0=src[c][:, n0:n0 + nn],
                                         in1=src[c][:, n0:n0 + nn])
                else:
                    nc.scalar.activation(out=sq, in_=src[c][:, n0:n0 + nn],
                                         func=AF.Square)
                nc.tensor.matmul(out=ssq, lhsT=ones, rhs=sq,
                                 start=(c == 0), stop=(c == CT - 1))
            rstd = normp.tile([1, nn], F32, tag="rstd", name="rstd", bufs=2)
            nc.scalar.activation(out=rstd, in_=ssq, func=AF.Sqrt,
                                 scale=1.0 / C, bias=eps_t)
            nc.vector.reciprocal(out=rstd, in_=rstd)
            rbc = normp.tile([P, nn], F32, tag="rbc", name="rbc", bufs=2)
            nc.gpsimd.partition_broadcast(out_ap=rbc, in_ap=rstd)
            for c in range(CT):
                nc.vector.tensor_mul(out=out[c][:, n0:n0 + nn],
                                     in0=src[c][:, n0:n0 + nn], in1=rbc)
        return out

    def proj_fm(pool, wT, xn, ncols, otiles, tag, nchunk=512, order=None):
        out = [pool.tile([P, ncols], BF16, tag=f"{tag}_{o}", name=f"{tag}_{o}")
               for o in range(otiles)]
        for o in (order if order is not None else range(otiles)):
            wtb = wload_fused(wT, o)
            for n0 in range(0, ncols, nchunk):
                nn = min(nchunk, ncols - n0)
                ps = pspool.tile([P, nn], F32, tag="proj", name="proj", bufs=3)
                for c in range(CT):
                    nc.tensor.matmul(out=ps, lhsT=wtb[:, c, :],
                                     rhs=xn[c][:, n0:n0 + nn],
                                     start=(c == 0), stop=(c == CT - 1))
                nc.any.tensor_copy(out=out[o][:, n0:n0 + nn], in_=ps)
        return out

    def vproj_rm(pool, wT, xn, ttiles, tag):
        out = [pool.tile([P, C], BF16, tag=f"{tag}_{t}", name=f"{tag}_{t}")
               for t in range(ttiles)]
        for oc0 in range(0, C, 512):
            wts = []
            for c in range(CT):
                wt = wpool.tile([P, 512], BF16, tag="wv", name="wv", bufs=12)
                nc.sync.dma_start(out=wt, in_=wT[c * P:(c + 1) * P, oc0:oc0 + 512])
                wts.append(wt)
            for t in range(ttiles):
                ps = pspool.tile([P, 512], F32, tag="proj", name="proj", bufs=3)
                for c in range(CT):
                    nc.tensor.matmul(out=ps, lhsT=xn[c][:, t * P:(t + 1) * P],
                                     rhs=wts[c], start=(c == 0), stop=(c == CT - 1))
                nc.any.tensor_copy(out=out[t][:, oc0:oc0 + 512], in_=ps)
        return out

    def attention(pool, qT, kT, v, q_chunks, klen_of, mask_of, oT_tag,
                  ncols):
        """Transposed-scores attention. S^T[k,q] tiles are batched 4-per-PSUM
        bank so one exp covers [128, 4*qn_sub]; exp output (SBUF bf16) is
        directly the rhs of both the ones-matmul (denominators, free-major)
        and the O accumulation. Normalization happens at the output copy via
        partition-broadcast 1/den."""
        oT = [pool.tile([P, ncols], BF16, tag=f"{oT_tag}_{c}",
                        name=f"{oT_tag}_{c}") for c in range(CT)]
        for qi, (q0, qn) in enumerate(q_chunks):
            klen = klen_of(qi)
            nk = klen // P
            gsz = max(1, 512 // qn)              # k-tiles per PSUM bank
            mr = mask_of(qi)
            for hp in range(CT):                 # head pair = output c-tile
                po2 = pspool.tile([P, qn], F32, tag="ohead", name="ohead",
                                  bufs=2)
                for hh in range(2):
                    h = 2 * hp + hh
                    base = hh * 64
                    # pass 1: scores + exp, batched k-tiles per bank
                    pts = []
                    for g0 in range(0, nk, gsz):
                        gk = min(gsz, nk - g0)
                        ps = pspool.tile([P, gk, qn], F32, tag="scores",
                                         name="scores", bufs=3)
                        for j in range(gk):
                            kt = g0 + j
                            nc.tensor.matmul(
                                out=ps[:, j, :],
                                lhsT=kT[hp][base:base + 64,
                                            kt * P:(kt + 1) * P],
                                rhs=qT[hp][base:base + 64, q0:q0 + qn],
                                start=True, stop=True)
                            if mr is not None:
                                m0, m1 = mr
                                if m0 <= kt * P < m1:
                                    nc.vector.tensor_add(
                                        out=ps[:, j, :], in0=ps[:, j, :],
                                        in1=masks[qi][:, kt * P - m0:
                                                      (kt + 1) * P - m0])
                        pt = attnp.tile([P, gk, qn], BF16, tag="pt",
                                        name="pt", bufs=6)
                        nc.scalar.activation(out=pt, in_=ps, func=AF.Exp)
                        pts.append((g0, gk, pt))
                    # pass 2: denominator ladder, then O ladder
                    dps = pspool.tile([1, qn], F32, tag="proj", name="den",
                                      bufs=3)
                    for g0, gk, pt in pts:
                        for j in range(gk):
                            kt = g0 + j
                            nc.tensor.matmul(out=dps, lhsT=ones,
                                             rhs=pt[:, j, :],
                                             start=(kt == 0),
                                             stop=(kt == nk - 1))
                    for g0, gk, pt in pts:
                        for j in range(gk):
                            kt = g0 + j
                            nc.tensor.matmul(out=po2[base:base + 64, :],
                                             lhsT=v[kt][:, h * 64:
                                                        (h + 1) * 64],
                                             rhs=pt[:, j, :],
                                             tile_position=(0, base),
                                             start=(kt == 0),
                                             stop=(kt == nk - 1))
                    rdT = attnp.tile([1, qn], F32, tag="rdT", name="rdT",
                                     bufs=4)
                    nc.vector.reciprocal(out=rdT, in_=dps)
                    rbf = attnp.tile([P, qn], F32, tag="rb2", name="rbf",
                                     bufs=2)
                    nc.gpsimd.partition_broadcast(out_ap=rbf, in_ap=rdT,
                                                  channels=P)
                    nc.vector.tensor_mul(
                        out=oT[hp][base:base + 64, q0:q0 + qn],
                        in0=po2[base:base + 64, :], in1=rbf[0:64, :])

        return oT

    def wo_residual(wT, oT, res_in, res_out, ncols):
        for n0 in range(0, ncols, 512):
            nn = min(512, ncols - n0)
            for o in range(CT):
                wtb = wload_fused(wT, o)
                ps = pspool.tile([P, nn], F32, tag="proj", name="proj", bufs=3)
                for c in range(CT):
                    nc.tensor.matmul(out=ps, lhsT=wtb[:, c, :],
                                     rhs=oT[c][:, n0:n0 + nn],
                                     start=(c == 0), stop=(c == CT - 1))
                nc.vector.tensor_add(out=res_out[o][:, n0:n0 + nn], in0=ps,
                                     in1=res_in[o][:, n0:n0 + nn])

    # registers for own-token strip offsets (consumed by DVE dynamic APs)
    s0 = nc.vector.alloc_register("s0")
    nc.vector.reg_load(s0, stile[0:1, 0:1])
    s0v = nc.vector.snap(s0, donate=True, min_val=0, max_val=768)
    s1 = nc.vector.alloc_register("s1")
    nc.vector.reg_load(s1, stile[0:1, 1:2])
    s1v = nc.vector.snap(s1, donate=True, min_val=0, max_val=768)

    def gather_own(dst, src):
        nc.vector.tensor_copy(out=dst[:, 0:256], in_=src[:, ds(s0v, 256)])
        nc.vector.tensor_copy(out=dst[:, 256:512], in_=src[:, ds(s1v, 256)])

    half = CT // 2

    # ---- cross-attention (x updated in place to x') ----
    xpool = tc.alloc_tile_pool(name="x", bufs=1, side="right")
    x = [xpool.tile([P, T], F32, tag=f"x_{c}", name=f"x_{c}")
         for c in range(CT)]
    capool = tc.alloc_tile_pool(name="ca", bufs=1, side="right")
    y = [capool.tile([P, M], F32, tag=f"y_{c}", name=f"y_{c}")
         for c in range(CT)]
    for c in range(CT):
        nc.sync.dma_start(out=y[c], in_=io["yT"][c * P:(c + 1) * P, :])
    for half_c in range(2):
        for c in range(CT):
            nc.sync.dma_start(
                out=x[c][:, half_c * 512:(half_c + 1) * 512],
                in_=io["xT"][c * P:(c + 1) * P, half_c * 512:(half_c + 1) * 512])
    yn = rmsnorm(capool, y, M, "yn")
    xn0 = rmsnorm(capool, x, T, "xn0")
    kca = proj_fm(capool, io["ca_wkT"], yn, M, CT, "kca", nchunk=256)
    vca = vproj_rm(capool, io["ca_wvT"], yn, MT, "vca")
    qca = proj_fm(capool, io["ca_wqT"], xn0, T, CT, "qca")
    oca = attention(capool, qca, kca, vca, [(0, 512), (512, 512)],
                    lambda qi: M, lambda qi: None, "oca", T)
    wo_residual(io["ca_woT"], oca, x, x, T)
    capool.release()

    # ---- self-attention ----
    sa1 = tc.alloc_tile_pool(name="sa1", bufs=1)
    for c in range(CT):
        gather_own(xo[c], x[c])
    xn1 = rmsnorm(sa1, x, T, "xn1")
    xpool.release()

    xn1o = [sa1.tile([P, T_OWN], BF16, tag=f"xn1o_{c}", name=f"xn1o_{c}")
            for c in range(CT)]
    for c in range(CT):
        gather_own(xn1o[c], xn1[c])
    cso = sa1.tile([P, T_OWN], F32, tag="cso")
    sno = sa1.tile([P, T_OWN], F32, tag="sno")
    gather_own(cso, cs)
    gather_own(sno, sn)

    kvq = tc.alloc_tile_pool(name="kvq", bufs=1, side="right")
    ksa = [kvq.tile([P, T], BF16, tag=f"ksa_{c}", name=f"ksa_{c}")
           for c in range(CT)]
    qsa = [kvq.tile([P, T_OWN], BF16, tag=f"qsa_{c}", name=f"qsa_{c}")
           for c in range(CT)]

    sa2 = tc.alloc_tile_pool(name="sa2", bufs=1, side="right")
    qpre = proj_fm(sa2, io["sa_wqT"], xn1o, T_OWN, CT, "qpre",
                   order=[0, 4, 1, 5, 2, 6, 3, 7])
    kpre = proj_fm(sa2, io["sa_wkT"], xn1, T, CT, "kpre",
                   order=[0, 4, 1, 5, 2, 6, 3, 7])

    def rope_rearrange(pre, cc, ss, ncols, dst):
        # pre: global-deinterleaved projection tiles; writes per-head layout
        # into dst. Pair (t, t+half) -> heads 4t..4t+3.
        for t in range(half):
            otr = sa2.tile([P, ncols], BF16, tag="ror", name="ror", bufs=2)
            oti = sa2.tile([P, ncols], BF16, tag="roi", name="roi", bufs=2)
            for n0 in range(0, ncols, 512):
                nn = min(512, ncols - n0)
                sl = slice(n0, n0 + nn)
                tmp = sa2.tile([P, nn], F32, tag="ropetmp", name="ropetmp",
                               bufs=2)
                nc.vector.tensor_mul(out=otr[:, sl], in0=pre[t][:, sl],
                                     in1=cc[:, sl])
                nc.vector.tensor_mul(out=tmp, in0=pre[t + half][:, sl],
                                     in1=ss[:, sl])
                nc.vector.tensor_sub(out=otr[:, sl], in0=otr[:, sl], in1=tmp)
                tmp2 = sa2.tile([P, nn], F32, tag="ropetmp2", name="ropetmp2",
                                bufs=2)
                nc.vector.tensor_mul(out=oti[:, sl], in0=pre[t][:, sl],
                                     in1=ss[:, sl])
                nc.vector.tensor_mul(out=tmp2, in0=pre[t + half][:, sl],
                                     in1=cc[:, sl])
                nc.vector.tensor_add(out=oti[:, sl], in0=oti[:, sl], in1=tmp2)
            for hh in range(4):
                h = 4 * t + hh
                ct, base = h // 2, (h % 2) * 64
                nc.gpsimd.dma_start(out=dst[ct][base:base + 32, :],
                                    in_=otr[hh * 32:hh * 32 + 32, :])
                nc.gpsimd.dma_start(out=dst[ct][base + 32:base + 64, :],
                                    in_=oti[hh * 32:hh * 32 + 32, :])

    rope_rearrange(qpre, cso, sno, T_OWN, qsa)
    rope_rearrange(kpre, cs, sn, T, ksa)
    sa2.release()

    vsa = vproj_rm(kvq, io["sa_wvT"], xn1, TT, "vsa")
    sa1.release()
    osa = attention(kvq, qsa, ksa, vsa,
                    [(s_ * P, P) for s_ in range(OT)],
                    lambda qi: KLEN[qi], lambda qi: MASK_RANGE[qi], "osa",
                    T_OWN)
    wo_residual(io["sa_woT"], osa, xo, xo, T_OWN)
    kvq.release()
    attnp.release()

    # ---- SwiGLU MLP on own tokens ----
    mlpw = tc.alloc_tile_pool(name="mlpw", bufs=1)
    mpool = tc.alloc_tile_pool(name="mlp", bufs=1)
    xn2 = rmsnorm(mpool, xo, T_OWN, "xn2")
    ht = [mpool.tile([P, T_OWN], BF16, tag=f"h_{f}", name=f"h_{f}")
          for f in range(FT)]
    for f in range(FT):
        w1b = wload_fused(io["w_fc1T"], f)
        ps1 = pspool.tile([P, T_OWN], F32, tag="proj", name="proj", bufs=3)
        for c in range(CT):
            nc.tensor.matmul(out=ps1, lhsT=w1b[:, c, :], rhs=xn2[c],
                             start=(c == 0), stop=(c == CT - 1))
        s1t = mpool.tile([P, T_OWN], BF16, tag="silu", name="silu", bufs=3)
        nc.scalar.activation(out=s1t, in_=ps1, func=AF.Sigmoid)
        nc.vector.tensor_mul(out=s1t, in0=s1t, in1=ps1)
        w2b = wload_fused(io["w_fc2T"], f)
        ps2 = pspool.tile([P, T_OWN], F32, tag="proj", name="proj", bufs=3)
        for c in range(CT):
            nc.tensor.matmul(out=ps2, lhsT=w2b[:, c, :], rhs=xn2[c],
                             start=(c == 0), stop=(c == CT - 1))
        nc.vector.tensor_mul(out=ht[f], in0=s1t, in1=ps2)
    for o in range(CT):
        wpb = mlpw.tile([P, FT, P], BF16, tag="wpb", name="wpb", bufs=3)
        nc.sync.dma_start(
            out=wpb,
            in_=io["w_projT"][:, o * P:(o + 1) * P].rearrange(
                "(a p) o -> p a o", p=P))
        ps = pspool.tile([P, T_OWN], F32, tag="proj", name="proj", bufs=3)
        for f in range(FT):
            nc.tensor.matmul(out=ps, lhsT=wpb[:, f, :], rhs=ht[f],
                             start=(f == 0), stop=(f == FT - 1))
        ob = mpool.tile([P, T_OWN], F32, tag="ob", name="ob", bufs=3)
        nc.vector.tensor_add(out=ob, in0=ps, in1=xo[o])
        nc.sync.dma_start(out=io["outT"][o * P:(o + 1) * P, :], in_=ob)
    mpool.release()
    mlpw.release()
    xopool.release()
    normp.release()
    wpool.release()
    pspool.release()
    const.release()


def prep_inputs(inputs):
    """Host-side prep: transpose/permute/fold weights, build per-core maps."""
    g = {k: np.asarray(v) for k, v in inputs.items()}
    scale = 1.0 / np.sqrt(HD)
    g0, g0t, g1, g2 = g["ln0_s"], g["ln0t_s"], g["ln1_s"], g["ln2_s"]
    perm = rope_perm()

    shared = {
        "ca_wqT": _bf16(((g["ca_wq"] * scale) * g0[None, :]).T),
        "ca_wkT": _bf16((g["ca_wk"] * g0t[None, :]).T),
        "ca_wvT": _bf16((g["ca_wv"] * g0t[None, :]).T),
        "ca_woT": _bf16(g["ca_wo"].T),
        "sa_wqT": _bf16(((g["sa_wq"] * scale) * g1[None, :])[perm, :].T),
        "sa_wkT": _bf16((g["sa_wk"] * g1[None, :])[perm, :].T),
        "sa_wvT": _bf16((g["sa_wv"] * g1[None, :]).T),
        "sa_woT": _bf16(g["sa_wo"].T),
        "w_fc1T": _bf16((g["w_fc1"] * g2[None, :]).T),
        "w_fc2T": _bf16((g["w_fc2"] * g2[None, :]).T),
        "w_projT": _bf16(g["w_proj"].T),
    }
    cosT = _f32(g["cos"].T)   # [32, T]
    sinT = _f32(g["sin"].T)
    shared["cosrep"] = _f32(np.tile(cosT, (4, 1)))
    shared["sinrep"] = _f32(np.tile(sinT, (4, 1)))

    for nm in ["ca_bq", "ca_bk", "ca_bv", "ca_bo",
               "sa_bq", "sa_bk", "sa_bv", "sa_bo"]:
        assert not np.any(g[nm]), f"nonzero bias {nm} unsupported"
    assert bool(np.all(g["padding_mask"])), "padding_mask must be all ones"

    # per-half softmax masks: slot s covers own block qt=OWN_BLOCKS[h][s],
    # mask range MASK_RANGE[s] (global key cols m0:m1). additive fp32.
    masks = {}
    for h in (0, 1):
        sm = np.zeros((OT, P, 384), np.float32)
        for s in range(OT):
            qt = OWN_BLOCKS[h][s]
            m0, m1 = MASK_RANGE[s]
            for kt in range(m0 // P, m1 // P):
                blk = sm[s, :, kt * P - m0:(kt + 1) * P - m0]
                if kt > qt:
                    blk[:] = NEG
                elif kt == qt:
                    # transposed orientation: [keys, queries]
                    blk[:] = np.tril(np.full((P, P), NEG, np.float32), -1)
        masks[h] = sm

    x, y = _f32(g["x"]), _f32(g["y"])
    in_maps = []
    for core in range(N_CORES):
        b, h = core // 2, core % 2
        m = dict(shared)
        m["xT"] = _f32(x[b].T)
        m["yT"] = _f32(y[b].T)
        m["smask"] = masks[h]
        m["strips"] = np.asarray([STRIPS[h]], np.uint32)
        in_maps.append(m)
    return in_maps


def assemble_output(results, dtype):
    out = np.empty((B, T, C), np.float32)
    for core in range(N_CORES):
        b, h = core // 2, core % 2
        oT = results[core]["outT"]          # [C, T_OWN]
        for li, qt in enumerate(OWN_BLOCKS[h]):
            out[b, qt * P:(qt + 1) * P, :] = oT[:, li * P:(li + 1) * P].T
    return out.astype(dtype, copy=False)


def kernel(**inputs):
    if "nc" not in _CACHE:
        _CACHE["nc"] = build_program()
    nc = _CACHE["nc"]
    in_maps = prep_inputs(inputs)
    res = bass_utils.run_bass_kernel_spmd(nc, in_maps,
                                          core_ids=list(range(N_CORES)))
    return assemble_output(res.results, np.asarray(inputs["x"]).dtype)



# revision 1
# speedup vs baseline: 1.1112x; 1.1112x over previous
"""Trainium2 Bass kernel for a dense transformer block (cross-attn + RoPE
self-attn + SwiGLU MLP), SPMD over 8 NeuronCores.

Sharding: core = (batch, half). Each core processes one batch (B=4) and half
its tokens (balanced causal split: blocks {0,1,6,7} vs {2,3,4,5} of 8x128).
Cross-attention and self-attention K/V are computed for the full sequence on
both cores of a pair (cheap duplication, no collectives). Q / attention /
output-proj / MLP run only on the core's own 512 tokens.

All activations are feature-major [C, tokens]; matmuls run in bf16 with fp32
accumulation; the residual stream stays fp32. Host-side weight prep folds the
RMSNorm gammas and the attention scale into the weights, pre-transposes them,
and applies a rope-deinterleave permutation to the self-attn q/k weights.
The even/odd-core differences (softmax masks, own-token strip offsets) are
shipped as per-core data so a single SPMD program serves all 8 cores.
"""

import numpy as np
import ml_dtypes

import concourse.bacc as bacc
import concourse.bass as bass
import concourse.mybir as mybir
import concourse.tile as tile
from concourse import bass_utils
from concourse.bass import ds

F32 = mybir.dt.float32
BF16 = mybir.dt.bfloat16
AF = mybir.ActivationFunctionType
ALU = mybir.AluOpType

B, T, M, C, H, FF = 4, 1024, 256, 1024, 16, 4096
HD = C // H
EPS = 1e-5
N_CORES = 8
P = 128
CT = C // P            # 8 c-tiles
TT = T // P            # 8 token blocks
T_OWN = T // 2         # 512 own tokens per core
OT = T_OWN // P        # 4 own blocks
FT = FF // P           # 32 ff tiles
MT = M // P            # 2 memory tiles (cross keys)
NEG = -1e30

# Own token blocks per half (causally balanced: 1+2+7+8 == 3+4+5+6 == 18)
OWN_BLOCKS = {0: [0, 1, 6, 7], 1: [2, 3, 4, 5]}
# Own tokens as two contiguous 256-col strips (start offsets)
STRIPS = {0: [0, 768], 1: [256, 512]}
# Self-attn slots: identical shapes on both halves. Slot i processes own block
# OWN_BLOCKS[h][i] with klen = KLEN[i]; the shipped mask covers MASK_RANGE[i].
KLEN = [384, 512, 896, 1024]
MASK_RANGE = [(0, 384), (128, 512), (512, 896), (640, 1024)]

_CACHE = {}


def _bf16(a):
    return np.ascontiguousarray(a.astype(ml_dtypes.bfloat16))


def _f32(a):
    return np.ascontiguousarray(a.astype(np.float32))


def rope_perm():
    """Row permutation for self-attn q/k weights: per head, even hd indices
    first (rows h*32+j <- h*64+2j), all heads' real parts in rows 0:512,
    imag parts in rows 512:1024."""
    perm = np.zeros(C, dtype=np.int64)
    for h in range(H):
        for j in range(HD // 2):
            perm[h * (HD // 2) + j] = h * HD + 2 * j
            perm[C // 2 + h * (HD // 2) + j] = h * HD + 2 * j + 1
    return perm


def build_program():
    nc = bacc.Bacc("TRN2", target_bir_lowering=False, debug=False,
                   num_devices=N_CORES)

    def din(name, shape, dtype):
        return nc.dram_tensor(name, shape, dtype, kind="ExternalInput").ap()

    xT = din("xT", [C, T], F32)
    yT = din("yT", [C, M], F32)
    ca_wqT = din("ca_wqT", [C, C], BF16)
    ca_wkT = din("ca_wkT", [C, C], BF16)
    ca_wvT = din("ca_wvT", [C, C], BF16)
    ca_woT = din("ca_woT", [C, C], BF16)
    sa_wqT = din("sa_wqT", [C, C], BF16)
    sa_wkT = din("sa_wkT", [C, C], BF16)
    sa_wvT = din("sa_wvT", [C, C], BF16)
    sa_woT = din("sa_woT", [C, C], BF16)
    w_fc1T = din("w_fc1T", [C, FF], BF16)
    w_fc2T = din("w_fc2T", [C, FF], BF16)
    w_projT = din("w_projT", [FF, C], BF16)
    cosrep = din("cosrep", [P, T], F32)
    sinrep = din("sinrep", [P, T], F32)
    smask = din("smask", [OT, P, 384], F32)
    strips = din("strips", [1, 2], mybir.dt.uint32)
    outT = nc.dram_tensor("outT", [C, T_OWN], F32, kind="ExternalOutput").ap()

    with tile.TileContext(nc) as tc:
        _body(tc, locals())
    nc.compile()
    return nc


def _body(tc, io):
    nc = tc.nc

    # ---- global pools / constants ----
    const = tc.alloc_tile_pool(name="const", bufs=1)
    ones = const.tile([P, 1], BF16)
    nc.vector.memset(ones, 1.0)
    eps_t = const.tile([1, 1], F32)
    nc.vector.memset(eps_t, EPS)
    cs = const.tile([P, T], F32, tag="cos")
    sn = const.tile([P, T], F32, tag="sin")
    nc.gpsimd.dma_start(out=cs, in_=io["cosrep"])
    nc.gpsimd.dma_start(out=sn, in_=io["sinrep"])
    stile = const.tile([1, 2], mybir.dt.uint32)
    nc.gpsimd.dma_start(out=stile, in_=io["strips"])
    masks = [const.tile([P, 384], F32, tag=f"smask{s}", name=f"smask{s}")
             for s in range(OT)]
    for s in range(OT):
        nc.gpsimd.dma_start(out=masks[s], in_=io["smask"][s])

    wpool = tc.alloc_tile_pool(name="w", bufs=1)
    pspool = tc.alloc_tile_pool(name="ps", bufs=1, space="PSUM")
    normp = tc.alloc_tile_pool(name="normp", bufs=1)
    xopool = tc.alloc_tile_pool(name="xo", bufs=1)
    xo = [xopool.tile([P, T_OWN], F32, tag=f"xo_{c}", name=f"xo_{c}")
          for c in range(CT)]
    attnp = tc.alloc_tile_pool(name="attnp", bufs=1)

    def wtile():
        return wpool.tile([P, P], BF16, tag="wt", name="wt", bufs=40)

    def wload_fused(wT, o, ntiles=None, ocols=P):
        """One DMA for all CT contraction tiles of output cols [o*ocols, +ocols).
        Returns tile [P, ntiles, ocols]; lhsT for c-tile c is t[:, c, :]."""
        nt = CT if ntiles is None else ntiles
        wtb = wpool.tile([P, nt, ocols], BF16, tag=f"wtb{nt}_{ocols}",
                         name="wtb", bufs=8 if nt == CT and ocols == P else 2)
        src = wT[:, o * ocols:(o + 1) * ocols].rearrange(
            "(a p) o -> p a o", p=P)
        nc.sync.dma_start(out=wtb, in_=src)
        return wtb

    def rmsnorm(pool, src, ncols, tag):
        out = [pool.tile([P, ncols], BF16, tag=f"xn_{tag}_{c}",
                         name=f"xn_{tag}_{c}") for c in range(CT)]
        for n0 in range(0, ncols, 512):
            nn = min(512, ncols - n0)
            ssq = pspool.tile([1, nn], F32, tag="ohead", name="ssq", bufs=2)
            for c in range(CT):
                sq = normp.tile([P, nn], BF16, tag="sq", name="sq", bufs=4)
                if c % 2 == 0:
                    nc.vector.tensor_mul(out=sq, in0=src[c][:, n0:n0 + nn],
                                         in1=src[c][:, n0:n0 + nn])
                else:
                    nc.scalar.activation(out=sq, in_=src[c][:, n0:n0 + nn],
                                         func=AF.Square)
                nc.tensor.matmul(out=ssq, lhsT=ones, rhs=sq,
                                 start=(c == 0), stop=(c == CT - 1))
            rstd = normp.tile([1, nn], F32, tag="rstd", name="rstd", bufs=2)
            nc.scalar.activation(out=rstd, in_=ssq, func=AF.Sqrt,
                                 scale=1.0 / C, bias=eps_t)
            nc.vector.reciprocal(out=rstd, in_=rstd)
            rbc = normp.tile([P, nn], F32, tag="rbc", name="rbc", bufs=2)
            nc.gpsimd.partition_broadcast(out_ap=rbc, in_ap=rstd)
            for c in range(CT):
                nc.vector.tensor_mul(out=out[c][:, n0:n0 + nn],
                                     in0=src[c][:, n0:n0 + nn], in1=rbc)
        return out

    def proj_fm(pool, wT, xn, ncols, otiles, tag, nchunk=512, order=None):
        out = [pool.tile([P, ncols], BF16, tag=f"{tag}_{o}", name=f"{tag}_{o}")
               for o in range(otiles)]
        for o in (order if order is not None else range(otiles)):
            wtb = wload_fused(wT, o)
            for n0 in range(0, ncols, nchunk):
                nn = min(nchunk, ncols - n0)
                ps = pspool.tile([P, nn], F32, tag="proj", name="proj", bufs=3)
                for c in range(CT):
                    nc.tensor.matmul(out=ps, lhsT=wtb[:, c, :],
                                     rhs=xn[c][:, n0:n0 + nn],
                                     start=(c == 0), stop=(c == CT - 1))
                nc.any.tensor_copy(out=out[o][:, n0:n0 + nn], in_=ps)
        return out

    def vproj_rm(pool, wT, xn, ttiles, tag):
        out = [pool.tile([P, C], BF16, tag=f"{tag}_{t}", name=f"{tag}_{t}")
               for t in range(ttiles)]
        for oc0 in range(0, C, 512):
            wts = []
            for c in range(CT):
                wt = wpool.tile([P, 512], BF16, tag="wv", name="wv", bufs=12)
                nc.sync.dma_start(out=wt, in_=wT[c * P:(c + 1) * P, oc0:oc0 + 512])
                wts.append(wt)
            for t in range(ttiles):
                ps = pspool.tile([P, 512], F32, tag="proj", name="proj", bufs=3)
                for c in range(CT):
                    nc.tensor.matmul(out=ps, lhsT=xn[c][:, t * P:(t + 1) * P],
                                     rhs=wts[c], start=(c == 0), stop=(c == CT - 1))
                nc.any.tensor_copy(out=out[t][:, oc0:oc0 + 512], in_=ps)
        return out

    def attention(pool, qT, kT, v, q_chunks, klen_of, mask_of, oT_tag,
                  ncols):
        """Transposed-scores attention. S^T[k,q] tiles are batched 4-per-PSUM
        bank so one exp covers [128, 4*qn_sub]; exp output (SBUF bf16) is
        directly the rhs of both the ones-matmul (denominators, free-major)
        and the O accumulation. Normalization happens at the output copy via
        partition-broadcast 1/den."""
        oT = [pool.tile([P, ncols], BF16, tag=f"{oT_tag}_{c}",
                        name=f"{oT_tag}_{c}") for c in range(CT)]
        for qi, (q0, qn) in enumerate(q_chunks):
            klen = klen_of(qi)
            nk = klen // P
            gsz = max(1, 512 // qn)              # k-tiles per PSUM bank
            mr = mask_of(qi)
            for hp in range(CT):                 # head pair = output c-tile
                po2 = pspool.tile([P, qn], F32, tag="ohead", name="ohead",
                                  bufs=2)
                for hh in range(2):
                    h = 2 * hp + hh
                    base = hh * 64
                    # pass 1: scores + exp, batched k-tiles per bank
                    pts = []
                    for g0 in range(0, nk, gsz):
                        gk = min(gsz, nk - g0)
                        ps = pspool.tile([P, gk, qn], F32, tag="scores",
                                         name="scores", bufs=3)
                        for j in range(gk):
                            kt = g0 + j
                            nc.tensor.matmul(
                                out=ps[:, j, :],
                                lhsT=kT[hp][base:base + 64,
                                            kt * P:(kt + 1) * P],
                                rhs=qT[hp][base:base + 64, q0:q0 + qn],
                                start=True, stop=True)
                            if mr is not None:
                                m0, m1 = mr
                                if m0 <= kt * P < m1:
                                    nc.vector.tensor_add(
                                        out=ps[:, j, :], in0=ps[:, j, :],
                                        in1=masks[qi][:, kt * P - m0:
                                                      (kt + 1) * P - m0])
                        pt = attnp.tile([P, gk, qn], BF16, tag="pt",
                                        name="pt", bufs=6)
                        nc.scalar.activation(out=pt, in_=ps, func=AF.Exp)
                        pts.append((g0, gk, pt))
                    # pass 2: denominator ladder, then O ladder
                    dps = pspool.tile([1, qn], F32, tag="proj", name="den",
                                      bufs=3)
                    for g0, gk, pt in pts:
                        for j in range(gk):
                            kt = g0 + j
                            nc.tensor.matmul(out=dps, lhsT=ones,
                                             rhs=pt[:, j, :],
                                             start=(kt == 0),
                                             stop=(kt == nk - 1))
                    for g0, gk, pt in pts:
                        for j in range(gk):
                            kt = g0 + j
                            nc.tensor.matmul(out=po2[base:base + 64, :],
                                             lhsT=v[kt][:, h * 64:
                                                        (h + 1) * 64],
                                             rhs=pt[:, j, :],
                                             tile_position=(0, base),
                                             start=(kt == 0),
                                             stop=(kt == nk - 1))
                    rdT = attnp.tile([1, qn], F32, tag="rdT", name="rdT",
                                     bufs=4)
                    nc.vector.reciprocal(out=rdT, in_=dps)
                    rbf = attnp.tile([P, qn], F32, tag="rb2", name="rbf",
                                     bufs=2)
                    nc.gpsimd.partition_broadcast(out_ap=rbf, in_ap=rdT,
                                                  channels=P)
                    nc.vector.tensor_mul(
                        out=oT[hp][base:base + 64, q0:q0 + qn],
                        in0=po2[base:base + 64, :], in1=rbf[0:64, :])

        return oT

    def wo_residual(wT, oT, res_in, res_out, ncols):
        for n0 in range(0, ncols, 512):
            nn = min(512, ncols - n0)
            for o in range(CT):
                wtb = wload_fused(wT, o)
                ps = pspool.tile([P, nn], F32, tag="proj", name="proj", bufs=3)
                for c in range(CT):
                    nc.tensor.matmul(out=ps, lhsT=wtb[:, c, :],
                                     rhs=oT[c][:, n0:n0 + nn],
                                     start=(c == 0), stop=(c == CT - 1))
                nc.vector.tensor_add(out=res_out[o][:, n0:n0 + nn], in0=ps,
                                     in1=res_in[o][:, n0:n0 + nn])

    # registers for own-token strip offsets (consumed by DVE dynamic APs)
    s0 = nc.vector.alloc_register("s0")
    nc.vector.reg_load(s0, stile[0:1, 0:1])
    s0v = nc.vector.snap(s0, donate=True, min_val=0, max_val=768)
    s1 = nc.vector.alloc_register("s1")
    nc.vector.reg_load(s1, stile[0:1, 1:2])
    s1v = nc.vector.snap(s1, donate=True, min_val=0, max_val=768)

    def gather_own(dst, src):
        nc.vector.tensor_copy(out=dst[:, 0:256], in_=src[:, ds(s0v, 256)])
        nc.vector.tensor_copy(out=dst[:, 256:512], in_=src[:, ds(s1v, 256)])

    half = CT // 2

    # ---- cross-attention (x updated in place to x') ----
    xpool = tc.alloc_tile_pool(name="x", bufs=1, side="right")
    x = [xpool.tile([P, T], F32, tag=f"x_{c}", name=f"x_{c}")
         for c in range(CT)]
    capool = tc.alloc_tile_pool(name="ca", bufs=1, side="right")
    y = [capool.tile([P, M], F32, tag=f"y_{c}", name=f"y_{c}")
         for c in range(CT)]
    for c in range(CT):
        nc.sync.dma_start(out=y[c], in_=io["yT"][c * P:(c + 1) * P, :])
    for half_c in range(2):
        for c in range(CT):
            nc.sync.dma_start(
                out=x[c][:, half_c * 512:(half_c + 1) * 512],
                in_=io["xT"][c * P:(c + 1) * P, half_c * 512:(half_c + 1) * 512])
    yn = rmsnorm(capool, y, M, "yn")
    xn0 = rmsnorm(capool, x, T, "xn0")
    kca = proj_fm(capool, io["ca_wkT"], yn, M, CT, "kca", nchunk=256)
    vca = vproj_rm(capool, io["ca_wvT"], yn, MT, "vca")
    qca = proj_fm(capool, io["ca_wqT"], xn0, T, CT, "qca")
    oca = attention(capool, qca, kca, vca, [(0, 512), (512, 512)],
                    lambda qi: M, lambda qi: None, "oca", T)
    wo_residual(io["ca_woT"], oca, x, x, T)
    capool.release()

    # ---- self-attention ----
    sa1 = tc.alloc_tile_pool(name="sa1", bufs=1)
    for c in range(CT):
        gather_own(xo[c], x[c])
    xn1 = rmsnorm(sa1, x, T, "xn1")
    xpool.release()

    xn1o = [sa1.tile([P, T_OWN], BF16, tag=f"xn1o_{c}", name=f"xn1o_{c}")
            for c in range(CT)]
    for c in range(CT):
        gather_own(xn1o[c], xn1[c])
    cso = sa1.tile([P, T_OWN], F32, tag="cso")
    sno = sa1.tile([P, T_OWN], F32, tag="sno")
    gather_own(cso, cs)
    gather_own(sno, sn)

    kvq = tc.alloc_tile_pool(name="kvq", bufs=1, side="right")
    ksa = [kvq.tile([P, T], BF16, tag=f"ksa_{c}", name=f"ksa_{c}")
           for c in range(CT)]
    qsa = [kvq.tile([P, T_OWN], BF16, tag=f"qsa_{c}", name=f"qsa_{c}")
           for c in range(CT)]

    sa2 = tc.alloc_tile_pool(name="sa2", bufs=1, side="right")
    qpre = proj_fm(sa2, io["sa_wqT"], xn1o, T_OWN, CT, "qpre",
                   order=[0, 4, 1, 5, 2, 6, 3, 7])
    kpre = proj_fm(sa2, io["sa_wkT"], xn1, T, CT, "kpre",
                   order=[0, 4, 1, 5, 2, 6, 3, 7])

    def rope_rearrange(pre, cc, ss, ncols, dst):
        # pre: global-deinterleaved projection tiles; writes per-head layout
        # into dst. Pair (t, t+half) -> heads 4t..4t+3.
        for t in range(half):
            otr = sa2.tile([P, ncols], BF16, tag="ror", name="ror", bufs=2)
            oti = sa2.tile([P, ncols], BF16, tag="roi", name="roi", bufs=2)
            for n0 in range(0, ncols, 512):
                nn = min(512, ncols - n0)
                sl = slice(n0, n0 + nn)
                tmp = sa2.tile([P, nn], F32, tag="ropetmp", name="ropetmp",
                               bufs=2)
                nc.vector.tensor_mul(out=otr[:, sl], in0=pre[t][:, sl],
                                     in1=cc[:, sl])
                nc.vector.tensor_mul(out=tmp, in0=pre[t + half][:, sl],
                                     in1=ss[:, sl])
                nc.vector.tensor_sub(out=otr[:, sl], in0=otr[:, sl], in1=tmp)
                tmp2 = sa2.tile([P, nn], F32, tag="ropetmp2", name="ropetmp2",
                                bufs=2)
                nc.vector.tensor_mul(out=oti[:, sl], in0=pre[t][:, sl],
                                     in1=ss[:, sl])
                nc.vector.tensor_mul(out=tmp2, in0=pre[t + half][:, sl],
                                     in1=cc[:, sl])
                nc.vector.tensor_add(out=oti[:, sl], in0=oti[:, sl], in1=tmp2)
            for hh in range(4):
                h = 4 * t + hh
                ct, base = h // 2, (h % 2) * 64
                nc.gpsimd.dma_start(out=dst[ct][base:base + 32, :],
                                    in_=otr[hh * 32:hh * 32 + 32, :])
                nc.gpsimd.dma_start(out=dst[ct][base + 32:base + 64, :],
                                    in_=oti[hh * 32:hh * 32 + 32, :])

    rope_rearrange(qpre, cso, sno, T_OWN, qsa)
    rope_rearrange(kpre, cs, sn, T, ksa)
    sa2.release()

    vsa = vproj_rm(kvq, io["sa_wvT"], xn1, TT, "vsa")
    sa1.release()
    osa = attention(kvq, qsa, ksa, vsa,
                    [(s_ * P, P) for s_ in range(OT)],
                    lambda qi: KLEN[qi], lambda qi: MASK_RANGE[qi], "osa",
                    T_OWN)
    wo_residual(io["sa_woT"], osa, xo, xo, T_OWN)
    kvq.release()
    attnp.release()

    # ---- SwiGLU MLP on own tokens ----
    mlpw = tc.alloc_tile_pool(name="mlpw", bufs=1)
    mpool = tc.alloc_tile_pool(name="mlp", bufs=1)
    xn2 = rmsnorm(mpool, xo, T_OWN, "xn2")
    ht = [mpool.tile([P, T_OWN], BF16, tag=f"h_{f}", name=f"h_{f}")
          for f in range(FT)]
    for f in range(FT):
        w1b = wload_fused(io["w_fc1T"], f)
        ps1 = pspool.tile([P, T_OWN], F32, tag="proj", name="proj", bufs=3)
        for c in range(CT):
            nc.tensor.matmul(out=ps1, lhsT=w1b[:, c, :], rhs=xn2[c],
                             start=(c == 0), stop=(c == CT - 1))
        s1t = mpool.tile([P, T_OWN], BF16, tag="silu", name="silu", bufs=3)
        nc.scalar.activation(out=s1t, in_=ps1, func=AF.Sigmoid)
        nc.vector.tensor_mul(out=s1t, in0=s1t, in1=ps1)
        w2b = wload_fused(io["w_fc2T"], f)
        ps2 = pspool.tile([P, T_OWN], F32, tag="proj", name="proj", bufs=3)
        for c in range(CT):
            nc.tensor.matmul(out=ps2, lhsT=w2b[:, c, :], rhs=xn2[c],
                             start=(c == 0), stop=(c == CT - 1))
        nc.vector.tensor_mul(out=ht[f], in0=s1t, in1=ps2)
    for o in range(CT):
        wpb = mlpw.tile([P, FT, P], BF16, tag="wpb", name="wpb", bufs=3)
        nc.sync.dma_start(
            out=wpb,
            in_=io["w_projT"][:, o * P:(o + 1) * P].rearrange(
                "(a p) o -> p a o", p=P))
        ps = pspool.tile([P, T_OWN], F32, tag="proj", name="proj", bufs=3)
        for f in range(FT):
            nc.tensor.matmul(out=ps, lhsT=wpb[:, f, :], rhs=ht[f],
                             start=(f == 0), stop=(f == FT - 1))
        ob = mpool.tile([P, T_OWN], F32, tag="ob", name="ob", bufs=3)
        nc.vector.tensor_add(out=ob, in0=ps, in1=xo[o])
        nc.sync.dma_start(out=io["outT"][o * P:(o + 1) * P, :], in_=ob)
    mpool.release()
    mlpw.release()
    xopool.release()
    normp.release()
    wpool.release()
    pspool.release()
    const.release()


def prep_inputs(inputs):
    """Host-side prep: transpose/permute/fold weights, build per-core maps."""
    g = {k: np.asarray(v) for k, v in inputs.items()}
    scale = 1.0 / np.sqrt(HD)
    g0, g0t, g1, g2 = g["ln0_s"], g["ln0t_s"], g["ln1_s"], g["ln2_s"]
    perm = rope_perm()

    shared = {
        "ca_wqT": _bf16(((g["ca_wq"] * scale) * g0[None, :]).T),
        "ca_wkT": _bf16((g["ca_wk"] * g0t[None, :]).T),
        "ca_wvT": _bf16((g["ca_wv"] * g0t[None, :]).T),
        "ca_woT": _bf16(g["ca_wo"].T),
        "sa_wqT": _bf16(((g["sa_wq"] * scale) * g1[None, :])[perm, :].T),
        "sa_wkT": _bf16((g["sa_wk"] * g1[None, :])[perm, :].T),
        "sa_wvT": _bf16((g["sa_wv"] * g1[None, :]).T),
        "sa_woT": _bf16(g["sa_wo"].T),
        "w_fc1T": _bf16((g["w_fc1"] * g2[None, :]).T),
        "w_fc2T": _bf16((g["w_fc2"] * g2[None, :]).T),
        "w_projT": _bf16(g["w_proj"].T),
    }
    cosT = _f32(g["cos"].T)   # [32, T]
    sinT = _f32(g["sin"].T)
    shared["cosrep"] = _f32(np.tile(cosT, (4, 1)))
    shared["sinrep"] = _f32(np.tile(sinT, (4, 1)))

    for nm in ["ca_bq", "ca_bk", "ca_bv", "ca_bo",
               "sa_bq", "sa_bk", "sa_bv", "sa_bo"]:
        assert not np.any(g[nm]), f"nonzero bias {nm} unsupported"
    assert bool(np.all(g["padding_mask"])), "padding_mask must be all ones"

    # per-half softmax masks: slot s covers own block qt=OWN_BLOCKS[h][s],
    # mask range MASK_RANGE[s] (global key cols m0:m1). additive fp32.
    masks = {}
    for h in (0, 1):
        sm = np.zeros((OT, P, 384), np.float32)
        for s in range(OT):
            qt = OWN_BLOCKS[h][s]
            m0, m1 = MASK_RANGE[s]
            for kt in range(m0 // P, m1 // P):
                blk = sm[s, :, kt * P - m0:(kt + 1) * P - m0]
                if kt > qt:
                    blk[:] = NEG
                elif kt == qt:
                    # transposed orientation: [keys, queries]
                    blk[:] = np.tril(np.full((P, P), NEG, np.float32), -1)
        masks[h] = sm

    x, y = _f32(g["x"]), _f32(g["y"])
    in_maps = []
    for core in range(N_CORES):
        b, h = core // 2, core % 2
        m = dict(shared)
        m["xT"] = _f32(x[b].T)
        m["yT"] = _f32(y[b].T)
        m["smask"] = masks[h]
        m["strips"] = np.asarray([STRIPS[h]], np.uint32)
        in_maps.append(m)
    return in_maps


def assemble_output(results, dtype):
    out = np.empty((B, T, C), np.float32)
    for core in range(N_CORES):
        b, h = core // 2, core % 2
        oT = results[core]["outT"]          # [C, T_OWN]
        for li, qt in enumerate(OWN_BLOCKS[h]):
            out[b, qt * P:(qt + 1) * P, :] = oT[:, li * P:(li + 1) * P].T
    return out.astype(dtype, copy=False)


def kernel(**inputs):
    if "nc" not in _CACHE:
        _CACHE["nc"] = build_program()
    nc = _CACHE["nc"]
    in_maps = prep_inputs(inputs)
    res = bass_utils.run_bass_kernel_spmd(nc, in_maps,
                                          core_ids=list(range(N_CORES)))
    return assemble_output(res.results, np.asarray(inputs["x"]).dtype)

